# revision 1
# baseline (speedup 1.0000x reference)
"""Trainium2 Bass kernel for nn_AttentionLayer_47596827574368.

Reference computation (per batch sample b, B=8, C=768, H=W=64, L=4096, Cqk=Cv=96):
  Q = Wq @ X, K = Wk @ X, V = Wv @ X            (X = x[b] as [C, L])
  S = Q^T K   [L, L];  beta = softmax(S, axis=-1)
  O = beta @ V^T      [L, Cv]
  y = gamma * (Wlast @ O^T) + X                 [C, L]

Sharding: data-parallel over batch — one sample per NeuronCore (8 cores).

Device plan (per core):
  X streamed in (chunk, 512-col) pieces; Q/K c-major [96, 4096]; V^T as 32
  blocks [128(k), 97] (col 96 = ones -> softmax denominators ride along in
  the attnV matmul); scores computed transposed S^T[k, q] per 128-k block.
  Softmax uses a global-shift exp (C = est_max + 8 sampled from k-block 0;
  exact per-row max is unnecessary: softmax is shift-invariant and fp32 exp
  has huge dynamic-range headroom). Normalization is applied before the
  final projection; gamma is folded into Wlast on the host.

  q columns are processed in chunks [512, 1024, 1024, 1024, 512]: the first
  512 chunk is interleaved with the projection phase (so ScalarE exps run
  under the input-DMA/projection span), the last 512 chunk keeps the final
  drain short. Each chunk's final-projection/residual work is spread
  through the next chunk's k loop. Matmuls run in float32r (full PE rate,
  ~1.5e-4 rel err). PSUM pools are phase-scoped: common 2 banks + 6 banks
  for either (proj + chunk-0) or (scores + accumulator).
"""

import numpy as np

import concourse.bass as bass
import concourse.tile as tile
import concourse.mybir as mybir
from concourse import bacc
from concourse import bass_utils
from concourse.masks import make_identity

F32 = mybir.dt.float32
F32R = mybir.dt.float32r
BF16 = mybir.dt.bfloat16
AF = mybir.ActivationFunctionType
AX = mybir.AxisListType

C = 768          # input/output channels
CQ = 96          # qk/v channels
L = 4096         # H*W
KC = C // 128    # 6 contraction chunks
NKB = L // 128   # 32 k blocks
MARGIN = 8.0     # exp shift safety margin

# q-column chunks: (start, width). Chunk 0 is PSUM-limited to 512 while
# interleaved with projections (its exps ride free under the DMA/projection
# span); the last 512 chunk keeps the final drain short.
CHUNKS = [(0, 512), (512, 1024), (1536, 1024), (2560, 1024), (3584, 512)]


def pieces(w):
    # split a chunk width into matmul-sized pieces (<=512, >=256 so f32r
    # stays at full rate and no PSUM bank is crossed)
    out = []
    off = 0
    while w - off > 512:
        out.append((off, 512))
        off += 512
    out.append((off, w - off))
    return out


def body(nc, tc, sbuf, x, wqkv_t, wl_t, y):
    # ---- persistent sbuf tiles -----------------------------------------
    # weights first: tiny, and everything depends on them
    w_sb = sbuf.tile([128, KC, 3 * CQ], F32R, tag="w")
    nc.sync.dma_start(
        out=w_sb, in_=wqkv_t.rearrange("(ko ki) m -> ki ko m", ki=128).bitcast(F32R)
    )
    wl_sb = sbuf.tile([CQ, C], F32R, tag="wl")
    nc.sync.dma_start(out=wl_sb, in_=wl_t.bitcast(F32R))

    x_sb = sbuf.tile([128, KC, L], F32R, tag="x")
    x_r = x.rearrange("(ko ki) l -> ki ko l", ki=128).bitcast(F32R)
    # stream X in (column-group, kc) pieces, in consumption order
    for gp in range(8):
        gs = slice(gp * 512, (gp + 1) * 512)
        for kc in range(KC):
            nc.sync.dma_start(out=x_sb[:, kc, gs], in_=x_r[:, kc, gs])

    ident = sbuf.tile([128, 128], F32, tag="ident")
    make_identity(nc, ident)
    ident_bf = sbuf.tile([128, 128], BF16, tag="identbf")
    make_identity(nc, ident_bf)

    q_sb = sbuf.tile([CQ, L], F32R, tag="q")
    k_sb = sbuf.tile([CQ, L], F32R, tag="k")
    v_sb = sbuf.tile([CQ, L], BF16, tag="vbig")
    vt_sb = sbuf.tile([128, NKB, CQ + 1], BF16, tag="vt")
    # ones column (f32r producer required: memset can't write f32r)
    nc.scalar.activation(
        out=vt_sb[:, :, CQ : CQ + 1].rearrange("p a b -> p (a b)"),
        in_=ident[:, 0:NKB],
        func=AF.Copy,
        bias=1.0,
        scale=0.0,
    )

    small = sbuf.tile([128, 16], F32, tag="small")
    m_part = small[:, 0:8]
    m_row = small[:, 8:9]
    neg_c = small[:, 9:10]
    gmax_bc = small[:, 10:11]
    mt_sb = sbuf.tile([1, 128], F32, tag="rcp")

    attn_sb = sbuf.tile([CQ, L], F32R, tag="vbig", name="attn_sb")
    rcp_bc = sbuf.tile([CQ, 1024], F32, tag="rbc")
    y_r = y.rearrange("(ko ki) l -> ki ko l", ki=128)

    def scores_mms(s_ps, kb, c0, w):
        for off, pw in pieces(w):
            nc.tensor.matmul(
                s_ps[:, off : off + pw],
                k_sb[:, kb * 128 : (kb + 1) * 128],
                q_sb[:, c0 + off : c0 + off + pw],
                start=True,
                stop=True,
            )

    def attnv_mms(out_ps, et, kb, w):
        for off, pw in pieces(w):
            nc.tensor.matmul(
                out_ps[0 : CQ + 1, off : off + pw],
                vt_sb[:, kb, :],
                et[:, off : off + pw],
                start=(kb == 0),
                stop=(kb == NKB - 1),
            )

    def normalize(ci, out_ps):
        # stage out_ps to SBUF in one copy so its PSUM banks free fast, then
        #   attn[:, c0:c0+w] = stage[0:96] * (1 / stage[96])
        c0, w = CHUNKS[ci]
        ostage = sbuf.tile([CQ + 1, 1024], F32, tag="ostage", bufs=1,
                           name=f"ostage_{ci}")[:, 0:w]
        nc.vector.tensor_copy(ostage, out_ps[0 : CQ + 1, 0:w])
        rcp_sb = sbuf.tile([1, 1024], F32, tag="rcp", name=f"rcp_{ci}")[:, 0:w]
        nc.vector.reciprocal(rcp_sb, ostage[CQ : CQ + 1, :])
        nc.gpsimd.partition_broadcast(rcp_bc[:, 0:w], rcp_sb)
        nc.vector.tensor_mul(attn_sb[:, c0 : c0 + w], ostage[0:CQ, :], rcp_bc[:, 0:w])

    def phase4_unit(ps_pool, ci, oc):
        # final projection + residual for one 128-row output chunk; z tiles
        # share the accumulator tag's slots (transient between long-lived
        # accumulator lifetimes)
        c0, w = CHUNKS[ci]
        y_sb = sbuf.tile([128, 1024], F32, tag="y", bufs=3,
                         name=f"y_sb_{ci}_{oc}")[:, 0:w]
        for g, (off, pw) in enumerate(pieces(w)):
            gs = slice(c0 + off, c0 + off + pw)
            z_ps = ps_pool.tile([128, pw], F32, tag="obig", bufs=2,
                                name=f"z_ps_{ci}_{oc}_{g}")
            nc.tensor.matmul(
                z_ps,
                wl_sb[:, oc * 128 : (oc + 1) * 128],
                attn_sb[:, gs],
                start=True,
                stop=True,
            )
            nc.vector.tensor_add(y_sb[:, off : off + pw], z_ps,
                                 x_sb[:, oc, gs].bitcast(F32))
        nc.sync.dma_start(out=y_r[:, oc, slice(c0, c0 + w)], in_=y_sb)

    # ---- phase 1 + attention chunk 0 (512 wide), interleaved ------------
    # projections run in 512-column groups; as each group's K/V land, the
    # corresponding k-blocks of chunk 0 are scored/exp'd/accumulated.
    with (
        tc.tile_pool(name="ps_proj", bufs=1, space="PSUM") as ps_proj,
        tc.tile_pool(name="ps_aux", bufs=2, space="PSUM") as ps_aux,
    ):
        out0_ps = ps_proj.tile([128, 512], F32, tag="o0", name="out0_ps")
        pend_attnv = []  # two-kb lag FIFO so PE never waits on ACT in-order
        for gp in range(8):
            gs = slice(gp * 512, (gp + 1) * 512)
            tiles = [
                ps_proj.tile([CQ, 512], F32, tag=f"proj{t}", name=f"p_ps_{t}_{gp}")
                for t in range(3)
            ]
            for kc in range(KC):
                for t in range(3):
                    nc.tensor.matmul(
                        tiles[t],
                        w_sb[:, kc, t * CQ : (t + 1) * CQ],
                        x_sb[:, kc, gs],
                        start=(kc == 0),
                        stop=(kc == KC - 1),
                    )
            for t, dst in ((0, q_sb), (1, k_sb), (2, v_sb)):
                if t == 1:
                    nc.vector.tensor_copy(dst[:, gs], tiles[t])
                else:
                    nc.scalar.copy(dst[:, gs], tiles[t])


            # V -> V^T transposes for this group's 4 l-blocks
            for lb in range(4 * gp, 4 * gp + 4):
                t_ps = ps_aux.tile([128, CQ], BF16, tag="sm", name=f"t_ps_{lb}")
                nc.tensor.transpose(
                    t_ps, v_sb[:, lb * 128 : (lb + 1) * 128], ident_bf[0:CQ, 0:CQ]
                )
                nc.vector.tensor_copy(vt_sb[:, lb, 0:CQ], t_ps)

            # chunk-0 attention for this group's 4 k-blocks
            for kb in range(4 * gp, 4 * gp + 4):
                s_ps = ps_proj.tile([128, 512], F32, tag="s0", bufs=2,
                                    name=f"s_ps_0_{kb}")
                scores_mms(s_ps, kb, 0, 512)
                if kb == 0:
                    # shift estimate from these 65k scores (statistically
                    # ample for a shift that merely has to land within
                    # ~+-80 of the true max)
                    nc.vector.reduce_max(m_row, s_ps, axis=AX.X)
                    mt_ps = ps_aux.tile([1, 128], F32, tag="sm")
                    nc.tensor.transpose(mt_ps, m_row, ident)
                    nc.vector.tensor_copy(mt_sb[:, 0:128], mt_ps)
                    nc.vector.reduce_max(small[0:1, 11:12], mt_sb[:, 0:128],
                                         axis=AX.X)
                    nc.gpsimd.partition_broadcast(gmax_bc, small[0:1, 11:12])
                    # neg_c = -(gmax + MARGIN)
                    nc.scalar.activation(neg_c, gmax_bc, AF.Copy,
                                         bias=-MARGIN, scale=-1.0)
                et = sbuf.tile([128, 1024], BF16, tag="et", bufs=4,
                               name=f"et_0_{kb}")[:, 0:512]
                nc.scalar.activation(et, s_ps, AF.Exp, bias=neg_c, scale=1.0)
                if len(pend_attnv) >= 2:
                    pa = pend_attnv.pop(0)
                    attnv_mms(out0_ps, pa[0], pa[1], 512)
                pend_attnv.append((et, kb))
        for pa in pend_attnv:
            attnv_mms(out0_ps, pa[0], pa[1], 512)
        # bridge: score+exp chunk-1's k-block 0 in this pool's slots so
        # ScalarE never idles across the PSUM pool swap
        bridge_et = sbuf.tile([128, 1024], BF16, tag="et", bufs=4, name="et_1_0")
        for h in range(2):
            sb_ps = ps_proj.tile([128, 512], F32, tag="s0", bufs=2,
                                 name=f"sb_ps_{h}")
            nc.tensor.matmul(
                sb_ps, k_sb[:, 0:128],
                q_sb[:, 512 + h * 512 : 512 + (h + 1) * 512],
                start=True, stop=True,
            )
            nc.scalar.activation(bridge_et[:, h * 512 : (h + 1) * 512], sb_ps,
                                 AF.Exp, bias=neg_c, scale=1.0)
        normalize(0, out0_ps)

    # ---- attention chunks 1..4 ------------------------------------------
    with tc.tile_pool(name="ps_attn", bufs=1, space="PSUM") as ps_attn:
        for ci in range(1, len(CHUNKS)):
            c0, w = CHUNKS[ci]
            out_ps = ps_attn.tile(
                [128, 1024], F32, tag="obig", bufs=2, name=f"out_ps_{ci}"
            )
            # two-kb attnV lag FIFO: PE never waits on ACT in-order
            pend = [(bridge_et, 0)] if ci == 1 else []
            for kb in range(1 if ci == 1 else 0, NKB):
                s_ps = ps_attn.tile(
                    [128, 1024], F32, tag="sbig", bufs=2, name=f"s_ps_{ci}_{kb}"
                )[:, 0:w]
                scores_mms(s_ps, kb, c0, w)
                et = sbuf.tile([128, 1024], BF16, tag="et", bufs=4,
                               name=f"et_{ci}_{kb}")[:, 0:w]
                nc.scalar.activation(et, s_ps, AF.Exp, bias=neg_c, scale=1.0)
                if len(pend) >= 2:
                    pe = pend.pop(0)
                    attnv_mms(out_ps, pe[0], pe[1], w)
                pend.append((et, kb))
                # spread the previous chunk's phase 4 through this chunk's
                # k loop (keeps the DVE queue shallow so the boundary chain
                # is never stuck behind it)
                if kb % 4 == 3 and kb // 4 < KC:
                    phase4_unit(ps_attn, ci - 1, kb // 4)
            for pe in pend:
                attnv_mms(out_ps, pe[0], pe[1], w)
            normalize(ci, out_ps)

        # last chunk's phase 4
        for oc in range(KC):
            phase4_unit(ps_attn, len(CHUNKS) - 1, oc)


def build(loop_iters=1):
    nc = bacc.Bacc("TRN2", target_bir_lowering=False, debug=False, num_devices=8)
    x = nc.dram_tensor("x", [C, L], F32, kind="ExternalInput").ap()
    wqkv_t = nc.dram_tensor("wqkv_t", [C, 3 * CQ], F32, kind="ExternalInput").ap()
    wl_t = nc.dram_tensor("wl_t", [CQ, C], F32, kind="ExternalInput").ap()
    y = nc.dram_tensor("y", [C, L], F32, kind="ExternalOutput").ap()

    with tile.TileContext(nc) as tc:
        with tc.tile_pool(name="sbuf", bufs=1) as sbuf:
            if loop_iters > 1:
                engines = (
                    mybir.EngineType.PE,
                    mybir.EngineType.Activation,
                    mybir.EngineType.DVE,
                    mybir.EngineType.Pool,
                    mybir.EngineType.SP,
                )
                with tc.For_i(0, loop_iters, hint_engines=engines):
                    body(nc, tc, sbuf, x, wqkv_t, wl_t, y)
            else:
                body(nc, tc, sbuf, x, wqkv_t, wl_t, y)

    nc.compile()
    return nc


_cached_nc = None


def kernel(x, Wq, Wk, Wv, Wlast, gamma):
    global _cached_nc
    x = np.ascontiguousarray(np.asarray(x, dtype=np.float32))
    B = x.shape[0]
    assert B == 8 and x.shape[1:] == (C, 64, 64)
    wqkv_t = np.ascontiguousarray(
        np.concatenate([Wq, Wk, Wv], axis=0).T.astype(np.float32)
    )
    wl_t = np.ascontiguousarray(
        (np.asarray(Wlast, np.float32) * np.float32(np.asarray(gamma)[0])).T
    )

    if _cached_nc is None:
        _cached_nc = build()
    nc = _cached_nc

    in_maps = [
        {
            "x": np.ascontiguousarray(x[b].reshape(C, L)),
            "wqkv_t": wqkv_t,
            "wl_t": wl_t,
        }
        for b in range(B)
    ]
    res = bass_utils.run_bass_kernel_spmd(nc, in_maps, core_ids=list(range(B)))
    out = np.stack([res.results[b]["y"].reshape(C, 64, 64) for b in range(B)])
    return out.astype(np.float32)

